# revision 24
# baseline (speedup 1.0000x reference)
"""Distributed MQA causal attention for TRN2 (8 NeuronCores).

Sharding: 8 cores = 2 (batch) x 4 (head-group tensor parallel).
Core c handles batch b=c//4, head group g=c%4 (8 heads, o-slice of 1024).
KV projection is replicated within a batch group.  After attention, the
per-core attn^T chunks are AllGather-ed (groups of 4) and each core computes
a 1024-wide column slice of the output projection.

All operands are pre-transposed, cast to bf16, and PACKED INTO PARTITION-MAJOR
TILE LAYOUT on the host, so every device load is a plain DMA with 8-32KB
contiguous runs per partition.  Loads are spread across the two HWDGE queues
(sync + scalar); stores that wait on long-running compute (attn^T -> cc_in)
go on the gpsimd SWDGE queue so they never head-of-line-block a load.
All matmuls run in bf16 with f32 accumulation in PSUM.
"""

import numpy as np
import ml_dtypes

import concourse.bass as bass
import concourse.mybir as mybir
import concourse.tile as tile
from concourse import bacc
from concourse.bass_utils import run_bass_kernel_spmd
from concourse.masks import make_identity

# Problem shape (hardcoded; kernel.py must be self-contained).
B, T, D = 2, 2048, 4096
H, HD = 32, 128
NCORES, TPG = 8, 4
HL = H // TPG            # 8 local heads per core
OL = HL * HD             # 1024 local q/o dims per core
P = 128
TC = 512                 # t-chunk width (moving-dim of the big GEMMs)
NTC = T // TC            # 4
ND = D // P              # 32 contraction tiles for D
NT = T // P              # 16 k-tiles
HH = OL // 2             # 512 = 4 heads
TC2 = TC // 2            # 256: half-chunk granularity of the packed x layout
SCALE = float(1.0 / np.sqrt(HD))

BF16 = mybir.dt.bfloat16
F32 = mybir.dt.float32

_CACHE = {}
LAST_RESULT = None  # BassKernelResults of the most recent run (for test harness)


def build_nc():
    nc = bacc.Bacc(None, target_bir_lowering=False, num_devices=NCORES)

    # Host-packed bf16 operands (partition-major tile layout).
    xT_ext = nc.declare_dram_parameter("xT", [2 * NTC, P, ND, TC2], BF16, isOutput=False)
    wqT_ext = nc.declare_dram_parameter("wqT", [HL, P, ND, P], BF16, isOutput=False)
    wkT_ext = nc.declare_dram_parameter("wkT", [P, ND, HD], BF16, isOutput=False)
    wvT_ext = nc.declare_dram_parameter("wvT", [P, ND, HD], BF16, isOutput=False)
    woT_ext = nc.declare_dram_parameter("woT", [2, P, ND, HH], BF16, isOutput=False)
    bq_ext = nc.declare_dram_parameter("bq", [OL], F32, isOutput=False)
    bk_ext = nc.declare_dram_parameter("bk", [HD], F32, isOutput=False)
    bv_ext = nc.declare_dram_parameter("bv", [HD], F32, isOutput=False)
    bo_ext = nc.declare_dram_parameter("bo", [OL], BF16, isOutput=False)
    out_ext = nc.declare_dram_parameter("out", [T, OL], F32, isOutput=True)

    NPIECE = 4  # xT chunk load pieces per queue

    with tile.TileContext(nc) as tc:
        with (
            tc.tile_pool(name="consts", bufs=1) as consts,
            tc.tile_pool(name="wpool", bufs=2) as wpool,
            tc.tile_pool(name="wsmall", bufs=1) as wsmall,
            tc.tile_pool(name="slab", bufs=2) as slabp,
            tc.tile_pool(name="big", bufs=1) as bigp,
            tc.tile_pool(name="qtc", bufs=2) as qtcp,
            tc.tile_pool(name="atc", bufs=1) as atcp,
            tc.tile_pool(name="esb", bufs=3) as esbp,
            tc.tile_pool(name="tmp", bufs=3) as tmpp,
            tc.tile_pool(name="psum", bufs=1, space="PSUM") as psump,
            tc.tile_pool(name="dram", bufs=1, space="DRAM") as dram,
        ):
            # ---- Small constants
            ident = consts.tile([P, P], BF16)
            make_identity(nc, ident[:])
            # Diagonal causal 0/1 mask: mask0[x, y] = 1 if y >= x else 0.
            # Band S-tiles are column-trimmed so every one reduces to this.
            mask0 = consts.tile([P, TC], BF16, name="mask0")
            nc.gpsimd.memset(mask0[:], 1.0)
            nc.gpsimd.affine_select(
                out=mask0[:],
                in_=mask0[:],
                pattern=[[1, TC]],
                compare_op=mybir.AluOpType.is_ge,
                fill=0.0,
                base=0,
                channel_multiplier=-1,
            )

            def load_xT(xT, c):
                # Two 256-wide packed half-chunks; 2 nd-pieces per queue each.
                w = ND // 4
                for half in range(2):
                    for i in range(4):
                        eng = (nc.sync, nc.scalar)[i % 2]
                        lo = i * w
                        eng.dma_start(
                            xT[:, lo : lo + w, half * TC2 : (half + 1) * TC2],
                            xT_ext[2 * c + half, :, lo : lo + w, :],
                        )

            # Startup loads are spread over THREE queues (sync, scalar, and
            # the otherwise-idle gpsimd SWDGE) so the k-projection can start
            # within a few microseconds.  gpsimd is only safe here, before any
            # cc_in store is enqueued on it.
            wkT = wsmall.tile([P, ND, HD], BF16, name="wkT")
            wvT = wsmall.tile([P, ND, HD], BF16, name="wvT")
            nc.sync.dma_start(wkT[:], wkT_ext[:])
            nc.scalar.dma_start(wvT[:], wvT_ext[:])
            # Chunk 0 is processed as two 256-wide QKV passes so compute can
            # start on the first half while the rest of x / wq streams in.
            xT0 = slabp.tile([P, ND, TC], BF16, tag="slab", name="xT0")
            wq_lo = wpool.tile([P, ND, HH], BF16, tag="bigw", name="wq_lo")
            wq_hi = wpool.tile([P, ND, HH], BF16, tag="bigw", name="wq_hi")

            def _wq_load(h):
                eng = (nc.sync, nc.scalar)[h % 2]
                dst = wq_lo if h < 4 else wq_hi
                eng.dma_start(dst[:, :, (h % 4) * P : (h % 4 + 1) * P], wqT_ext[h])

            hnd = ND // 2
            for qi, eng in enumerate((nc.sync, nc.scalar)):
                lo = qi * hnd
                eng.dma_start(
                    xT0[:, lo : lo + hnd, 0:TC2], xT_ext[0, :, lo : lo + hnd, :]
                )
            _wq_load(0)
            _wq_load(1)
            for qi, eng in enumerate((nc.sync, nc.scalar)):
                lo = qi * hnd
                eng.dma_start(
                    xT0[:, lo : lo + hnd, TC2:TC], xT_ext[1, :, lo : lo + hnd, :]
                )
            for h in range(2, HL):
                _wq_load(h)

            bq_sb = consts.tile([P, HL], F32)
            nc.sync.dma_start(bq_sb[:], bq_ext[:].rearrange("(o p) -> p o", p=P))
            bk_sb = consts.tile([P, 1], F32)
            nc.sync.dma_start(bk_sb[:], bk_ext[:].rearrange("(o p) -> p o", p=P))
            bv_sb = consts.tile([P, 1], F32)
            nc.sync.dma_start(bv_sb[:], bv_ext[:].rearrange("(o p) -> p o", p=P))
            bo_row = consts.tile([1, OL], BF16, name="bo_row")
            nc.scalar.dma_start(bo_row[:], bo_ext[None, :])
            bo_bc = consts.tile([P, OL], BF16)
            nc.gpsimd.partition_broadcast(bo_bc[:], bo_row[:])


            # Persistent attention operands.
            kT = bigp.tile([P, T], BF16, name="kT")               # [hd, t]
            vaug = bigp.tile([P, NT, HD + 1], BF16, name="vaug")  # [tk, kt, 129]
            nc.vector.memset(vaug[:, :, HD : HD + 1], 1.0)

            # AllGather buffers, one per t-chunk, partition-major:
            # cc_in[c][p, h, t] = attnT_c[p, h, t]; gathered over the 4 cores.
            cc_in = [
                dram.tile([P, HL, TC], BF16, name=f"cc_in{c}") for c in range(NTC)
            ]
            cc_g = [
                dram.tile([TPG, P, HL, TC], BF16, name=f"cc_g{c}") for c in range(NTC)
            ]
            # Chunk 3 ships in two 4-head halves.
            cc_in3 = [dram.tile([P, 4, TC], BF16, name=f"cc_in3{i}") for i in range(2)]
            cc_g3 = [
                dram.tile([TPG, P, 4, TC], BF16, name=f"cc_g3{i}") for i in range(2)
            ]

            wo_d = {}  # dc -> [P, ND, HH] tile, loaded during chunk-3 QKV

            def outproj_mms(c, gT, ots):
                for tt in range(TC // P):
                    for dc in range(OL // TC):
                        ps = psump.tile(
                            [P, TC], F32, tag="tr", bufs=2, name=f"pso{c}_{tt}_{dc}"
                        )
                        for i, ot in enumerate(ots):
                            nc.tensor.matmul(
                                ps[:],
                                gT[:, ot, tt * P : (tt + 1) * P],
                                wo_d[dc][:, ot, :],
                                start=(i == 0),
                                stop=(i == ND - 1),
                            )
                        osb = tmpp.tile(
                            [P, TC], F32, tag="osb", bufs=2, name=f"osb{c}{tt}{dc}"
                        )
                        nc.vector.tensor_tensor(
                            osb[:],
                            ps[:],
                            bo_bc[:, dc * TC : (dc + 1) * TC],
                            mybir.AluOpType.add,
                        )
                        nc.sync.dma_start(
                            out_ext[
                                c * TC + tt * P : c * TC + (tt + 1) * P,
                                dc * TC : (dc + 1) * TC,
                            ],
                            osb[:],
                        )

            def emit_outproj3():
                c = NTC - 1
                gT = slabp.tile([P, ND, TC], BF16, tag="slab", name="gT3")
                # 8KB-contiguous per-partition loads: one per (half, rank).
                for half in range(2):
                    eng = nc.sync if half == 0 else nc.scalar
                    for r in range(TPG):
                        eng.dma_start(
                            gT[:, r * HL + half * 4 : r * HL + half * 4 + 4, :],
                            cc_g3[half][r],
                        )
                # Accumulate first-half heads first (they arrive earlier).
                ots = [ot for ot in range(ND) if ot % HL < 4] + [
                    ot for ot in range(ND) if ot % HL >= 4
                ]
                outproj_mms(c, gT, ots)

            def emit_outproj(c):
                gT = slabp.tile([P, ND, TC], BF16, tag="slab", name=f"gT{c}")
                for r in range(TPG):
                    eng = nc.sync if r % 2 == 0 else nc.scalar
                    eng.dma_start(gT[:, r * HL : (r + 1) * HL, :], cc_g[c][r])
                outproj_mms(c, gT, list(range(ND)))

            # ------------- per t-chunk: QKV proj + attention ------------------
            for c in range(NTC):
                if c == 0:
                    xT = xT0
                else:
                    xT = _CACHE[f"_xT{c}"]  # prefetched below

                qTc = qtcp.tile([P, HL, TC], BF16, tag="qTc", name=f"qTc{c}")

                # k first, then v, then q heads: k/v weights are tiny and land
                # first, so compute starts as early as possible.  Chunk 0 runs
                # as two 256-wide passes so the first half computes while the
                # second half of x and the q weights are still streaming in.
                def qkv_pass(t0, tw):
                    for ot in [HL, HL + 1] + list(range(HL)):
                        ps = psump.tile(
                            [P, tw], F32, tag="mm512", bufs=2,
                            name=f"psqkv{c}_{ot}_{t0}",
                        )
                        for dt in range(ND):
                            if ot < 4:
                                lhsT = wq_lo[:, dt, ot * P : (ot + 1) * P]
                            elif ot < HL:
                                lhsT = wq_hi[:, dt, (ot - 4) * P : (ot - 3) * P]
                            elif ot == HL:
                                lhsT = wkT[:, dt, :]
                            else:
                                lhsT = wvT[:, dt, :]
                            nc.tensor.matmul(
                                ps[:],
                                lhsT,
                                xT[:, dt, t0 : t0 + tw],
                                start=(dt == 0),
                                stop=(dt == ND - 1),
                            )
                        if ot < HL:
                            nc.vector.tensor_scalar_add(
                                qTc[:, ot, t0 : t0 + tw], ps[:], bq_sb[:, ot : ot + 1]
                            )
                        elif ot == HL:
                            nc.vector.tensor_scalar_add(
                                kT[:, c * TC + t0 : c * TC + t0 + tw], ps[:], bk_sb[:]
                            )
                        else:
                            vt = tmpp.tile(
                                [P, tw], BF16, tag="vt", bufs=1, name=f"vt{c}{t0}"
                            )
                            nc.vector.tensor_scalar_add(vt[:], ps[:], bv_sb[:])
                            pstv = psump.tile(
                                [P, tw], BF16, tag="tr", bufs=2, name=f"pstv{c}{t0}"
                            )
                            for j in range(tw // P):
                                nc.tensor.transpose(
                                    pstv[:, j * P : (j + 1) * P],
                                    vt[:, j * P : (j + 1) * P],
                                    ident[:],
                                )
                            nc.vector.tensor_copy(
                                vaug[
                                    :,
                                    c * 4 + t0 // P : c * 4 + (t0 + tw) // P,
                                    0:HD,
                                ],
                                pstv[:].rearrange("p (g t) -> p g t", g=tw // P),
                            )

                if c == 0:
                    qkv_pass(0, TC2)
                    qkv_pass(TC2, TC2)
                else:
                    qkv_pass(0, TC)

                if c == NTC - 1:
                    # wo halves reuse the wq buffers; the tile deps make each
                    # load start once the matching wq subtile is dead.
                    wo_d[0] = wpool.tile([P, ND, HH], BF16, tag="bigw", name="wo0")
                    wo_d[1] = wpool.tile([P, ND, HH], BF16, tag="bigw", name="wo1")
                    nc.sync.dma_start(wo_d[0][:], woT_ext[0])
                    nc.sync.dma_start(wo_d[1][:], woT_ext[1])

                if c + 1 < NTC:
                    # Prefetch next chunk's x before this chunk's attention so
                    # nothing with a long semaphore wait sits ahead of it.
                    xTn = slabp.tile([P, ND, TC], BF16, tag="slab", name=f"xT{c+1}")
                    load_xT(xTn, c + 1)
                    _CACHE[f"_xT{c+1}"] = xTn

                # Attention for all local heads on this q-chunk.  The per-head
                # normalize (DVE) + transpose (TensorE) chain is deferred into
                # the NEXT head's score phase so it never gates the PE.
                attnT_c = atcp.tile([P, HL, TC], BF16, tag="atc", name=f"attnT{c}")
                nkt = (c + 1) * (TC // P)  # causal: k-tiles 0..nkt-1

                def emit_psta(ph, sbs):
                    psta = psump.tile(
                        [P, 4 * P], BF16, tag="tr", bufs=2, name=f"psta{c}{ph}"
                    )
                    for qt in range(TC // P):
                        nc.tensor.transpose(
                            psta[:, qt * P : (qt + 1) * P], sbs[qt][:], ident[:]
                        )
                    nc.vector.tensor_copy(attnT_c[:, ph, :], psta[:])

                pending = None
                for h in range(HL):
                    att_ps = [
                        psump.tile(
                            [P, HD + 1], F32, tag="attn", bufs=4,
                            name=f"att{c}_{h}_{qt}",
                        )
                        for qt in range(TC // P)
                    ]
                    for kt in range(nkt):
                        # Band tiles only need columns tq >= (kt-4c)*128; after
                        # trimming, the causal pattern is always the diagonal.
                        off = (kt - (nkt - 4)) * P if kt >= nkt - 4 else 0
                        ne = TC - off
                        ps_s = psump.tile(
                            [P, TC], F32, tag="mm512", bufs=2, name=f"pss{c}_{h}_{kt}"
                        )
                        nc.tensor.matmul(
                            ps_s[:, :ne],
                            kT[:, kt * P : (kt + 1) * P],
                            qTc[:, h, off:TC],
                            start=True,
                            stop=True,
                        )
                        if kt == 1 and pending is not None:
                            # Deferred by a full k-tile so the previous head's
                            # DVE normalize is long done when the PE transposes.
                            emit_psta(*pending)
                            pending = None
                        es = esbp.tile([P, TC], BF16, tag="esb", name=f"es{c}_{h}_{kt}")
                        nc.scalar.activation(
                            es[:, :ne],
                            ps_s[:, :ne],
                            mybir.ActivationFunctionType.Exp,
                            scale=SCALE,
                        )
                        if kt >= nkt - 4:
                            # Zero weights where k > q (pure diagonal after trim).
                            nc.vector.tensor_tensor(
                                es[:, :ne], es[:, :ne], mask0[:, :ne],
                                mybir.AluOpType.mult,
                            )
                        for qt in range(TC // P):
                            tqi = c * (TC // P) + qt
                            if kt > tqi:
                                continue
                            nc.tensor.matmul(
                                att_ps[qt][:],
                                es[:, qt * P - off : (qt + 1) * P - off],
                                vaug[:, kt, :],
                                start=(kt == 0),
                                stop=(kt == tqi),
                            )
                    sbs = []
                    for qt in range(TC // P):
                        recip = tmpp.tile([P, 1], F32, tag="recip", bufs=2, name=f"rc{c}{h}{qt}")
                        nc.vector.reciprocal(recip[:], att_ps[qt][:, HD : HD + 1])
                        attn_sb = tmpp.tile(
                            [P, P], BF16, tag="attn_sb", bufs=8, name=f"asb{c}{h}{qt}"
                        )
                        nc.vector.tensor_scalar_mul(
                            attn_sb[:], att_ps[qt][:, 0:HD], recip[:]
                        )
                        sbs.append(attn_sb)
                    pending = (h, sbs)
                    # Flush before anything that READS attnT_c (the cc_in
                    # stores at h==3 of the last chunk and at chunk end).
                    if h == HL - 1 or (c == NTC - 1 and h == 3):
                        emit_psta(*pending)
                        pending = None
                    if c == NTC - 1 and h == 3:
                        nc.gpsimd.dma_start(cc_in3[0][:], attnT_c[:, 0:4, :])
                        nc.gpsimd.collective_compute(
                            "AllGather",
                            mybir.AluOpType.bypass,
                            replica_groups=[[0, 1, 2, 3], [4, 5, 6, 7]],
                            ins=[cc_in3[0][:, :, :].opt()],
                            outs=[cc_g3[0][:, :, :, :].opt()],
                        )
                        emit_outproj(0)
                    if c == NTC - 1 and h == 5:
                        emit_outproj(1)
                    if c == NTC - 1 and h == 7:
                        emit_outproj(2)

                # Ship this chunk's attn^T and AllGather it within the group.
                if c < NTC - 1:
                    nc.gpsimd.dma_start(cc_in[c][:], attnT_c[:])
                    nc.gpsimd.collective_compute(
                        "AllGather",
                        mybir.AluOpType.bypass,
                        replica_groups=[[0, 1, 2, 3], [4, 5, 6, 7]],
                        ins=[cc_in[c][:, :, :].opt()],
                        outs=[cc_g[c][:, :, :, :].opt()],
                    )
                else:
                    nc.gpsimd.dma_start(cc_in3[1][:], attnT_c[:, 4:HL, :])
                    nc.gpsimd.collective_compute(
                        "AllGather",
                        mybir.AluOpType.bypass,
                        replica_groups=[[0, 1, 2, 3], [4, 5, 6, 7]],
                        ins=[cc_in3[1][:, :, :].opt()],
                        outs=[cc_g3[1][:, :, :, :].opt()],
                    )
                    emit_outproj3()

            for c in range(NTC):
                _CACHE.pop(f"_xT{c}", None)

    nc.compile()
    return nc


def _pack(mT, nd, width):
    """[nd*128, width] row-major -> [128, nd, width] partition-major."""
    return np.ascontiguousarray(mT.reshape(nd, P, width).transpose(1, 0, 2))


def kernel(x, wq_w, wq_b, wk_w, wk_b, wv_w, wv_b, wo_w, wo_b):
    global LAST_RESULT
    if "nc" not in _CACHE:
        _CACHE["nc"] = build_nc()
    nc = _CACHE["nc"]

    f32 = np.float32
    bf16 = ml_dtypes.bfloat16
    x = np.asarray(x, f32)
    wq_w = np.asarray(wq_w, f32)
    wk_w = np.asarray(wk_w, f32)
    wv_w = np.asarray(wv_w, f32)
    wo_w = np.asarray(wo_w, f32)

    # Host-side cast + transpose + pack into partition-major tile layout.
    # xT[c, p, nd, t] = x[b, c*TC+t, nd*P+p]
    xT_b = [
        np.ascontiguousarray(
            x[b].astype(bf16).reshape(2 * NTC, TC2, ND, P).transpose(0, 3, 2, 1)
        )
        for b in range(B)
    ]
    wkT = _pack(wk_w.astype(bf16).T, ND, HD)
    wvT = _pack(wv_w.astype(bf16).T, ND, HD)
    wqT_g = {}
    woT_g = {}
    for g in range(TPG):
        sl = slice(OL * g, OL * (g + 1))
        # wqT[h, p, nd, o] = wq[sl][h*P+o, nd*P+p]
        wqT_g[g] = np.ascontiguousarray(
            wq_w[sl].astype(bf16).reshape(HL, P, ND, P).transpose(0, 3, 2, 1)
        )
        # woT[half, p, nd, o] = wo[sl][half*HH+o, nd*P+p]
        woT_g[g] = np.ascontiguousarray(
            wo_w[sl].astype(bf16).reshape(2, HH, ND, P).transpose(0, 3, 2, 1)
        )

    in_maps = []
    for c in range(NCORES):
        b, g = divmod(c, TPG)
        sl = slice(OL * g, OL * (g + 1))
        in_maps.append(
            {
                "xT": xT_b[b],
                "wqT": wqT_g[g],
                "bq": np.ascontiguousarray(np.asarray(wq_b, f32)[sl]),
                "wkT": wkT,
                "bk": np.ascontiguousarray(np.asarray(wk_b, f32)),
                "wvT": wvT,
                "bv": np.ascontiguousarray(np.asarray(wv_b, f32)),
                "woT": woT_g[g],
                "bo": np.ascontiguousarray(np.asarray(wo_b, f32)[sl].astype(bf16)),
            }
        )

    import os

    trace = bool(os.environ.get("ATTN_TRACE"))
    res = run_bass_kernel_spmd(
        nc,
        in_maps,
        core_ids=list(range(NCORES)),
        trace=trace,
        trace_cores=list(range(NCORES)) if trace else None,
    )
    LAST_RESULT = res

    out = np.empty((B, T, D), dtype=f32)
    for c in range(NCORES):
        b, g = divmod(c, TPG)
        out[b, :, OL * g : OL * (g + 1)] = res.results[c]["out"]
    return out


# revision 27
# speedup vs baseline: 1.0077x; 1.0077x over previous
"""Distributed MQA causal attention for TRN2 (8 NeuronCores).

Sharding: 8 cores = 2 (batch) x 4 (head-group tensor parallel).
Core c handles batch b=c//4, head group g=c%4 (8 heads, o-slice of 1024).
KV projection is replicated within a batch group.  After attention, the
per-core attn^T chunks are AllGather-ed (groups of 4) and each core computes
a 1024-wide column slice of the output projection.

All operands are pre-transposed, cast to bf16, and PACKED INTO PARTITION-MAJOR
TILE LAYOUT on the host, so every device load is a plain DMA with 8-32KB
contiguous runs per partition.  Loads are spread across the two HWDGE queues
(sync + scalar); stores that wait on long-running compute (attn^T -> cc_in)
go on the gpsimd SWDGE queue so they never head-of-line-block a load.
All matmuls run in bf16 with f32 accumulation in PSUM.
"""

import numpy as np
import ml_dtypes

import concourse.bass as bass
import concourse.mybir as mybir
import concourse.tile as tile
from concourse import bacc
from concourse.bass_utils import run_bass_kernel_spmd
from concourse.masks import make_identity

# Problem shape (hardcoded; kernel.py must be self-contained).
B, T, D = 2, 2048, 4096
H, HD = 32, 128
NCORES, TPG = 8, 4
HL = H // TPG            # 8 local heads per core
OL = HL * HD             # 1024 local q/o dims per core
P = 128
TC = 512                 # t-chunk width (moving-dim of the big GEMMs)
NTC = T // TC            # 4
ND = D // P              # 32 contraction tiles for D
NT = T // P              # 16 k-tiles
HH = OL // 2             # 512 = 4 heads
SCALE = float(1.0 / np.sqrt(HD))

BF16 = mybir.dt.bfloat16
F32 = mybir.dt.float32

_CACHE = {}
LAST_RESULT = None  # BassKernelResults of the most recent run (for test harness)


def build_nc():
    nc = bacc.Bacc(None, target_bir_lowering=False, num_devices=NCORES)

    # Host-packed bf16 operands (partition-major tile layout).
    xT_ext = nc.declare_dram_parameter("xT", [NTC, P, ND, TC], BF16, isOutput=False)
    x0_ext = nc.declare_dram_parameter("x0", [2, P, ND, TC // 2], BF16, isOutput=False)
    wqT_ext = nc.declare_dram_parameter("wqT", [HL, P, ND, P], BF16, isOutput=False)
    wkT_ext = nc.declare_dram_parameter("wkT", [P, ND, HD], BF16, isOutput=False)
    wvT_ext = nc.declare_dram_parameter("wvT", [P, ND, HD], BF16, isOutput=False)
    woT_ext = nc.declare_dram_parameter("woT", [2, P, ND, HH], BF16, isOutput=False)
    bq_ext = nc.declare_dram_parameter("bq", [OL], F32, isOutput=False)
    bk_ext = nc.declare_dram_parameter("bk", [HD], F32, isOutput=False)
    bv_ext = nc.declare_dram_parameter("bv", [HD], F32, isOutput=False)
    bo_ext = nc.declare_dram_parameter("bo", [OL], BF16, isOutput=False)
    out_ext = nc.declare_dram_parameter("out", [T, OL], F32, isOutput=True)

    NPIECE = 4  # xT chunk load pieces per queue

    with tile.TileContext(nc) as tc:
        with (
            tc.tile_pool(name="consts", bufs=1) as consts,
            tc.tile_pool(name="wpool", bufs=2) as wpool,
            tc.tile_pool(name="wsmall", bufs=1) as wsmall,
            tc.tile_pool(name="slab", bufs=2) as slabp,
            tc.tile_pool(name="big", bufs=1) as bigp,
            tc.tile_pool(name="qtc", bufs=2) as qtcp,
            tc.tile_pool(name="atc", bufs=1) as atcp,
            tc.tile_pool(name="esb", bufs=3) as esbp,
            tc.tile_pool(name="tmp", bufs=3) as tmpp,
            tc.tile_pool(name="psum", bufs=1, space="PSUM") as psump,
            tc.tile_pool(name="dram", bufs=1, space="DRAM") as dram,
        ):
            # ---- Small constants
            ident = consts.tile([P, P], BF16)
            make_identity(nc, ident[:])
            # Diagonal causal 0/1 mask: mask0[x, y] = 1 if y >= x else 0.
            # Band S-tiles are column-trimmed so every one reduces to this.
            mask0 = consts.tile([P, TC], BF16, name="mask0")
            nc.gpsimd.memset(mask0[:], 1.0)
            nc.gpsimd.affine_select(
                out=mask0[:],
                in_=mask0[:],
                pattern=[[1, TC]],
                compare_op=mybir.AluOpType.is_ge,
                fill=0.0,
                base=0,
                channel_multiplier=-1,
            )

            def load_xT(xT, c):
                # 2*NPIECE interleaved pieces across both HWDGE queues.
                w = ND // (2 * NPIECE)
                for i in range(NPIECE):
                    for qi, eng in enumerate((nc.sync, nc.scalar)):
                        lo = (2 * i + qi) * w
                        eng.dma_start(
                            xT[:, lo : lo + w, :], xT_ext[c, :, lo : lo + w, :]
                        )

            # Startup loads are spread over THREE queues (sync, scalar, and
            # the otherwise-idle gpsimd SWDGE) so the k-projection can start
            # within a few microseconds.  gpsimd is only safe here, before any
            # cc_in store is enqueued on it.
            wkT = wsmall.tile([P, ND, HD], BF16, name="wkT")
            wvT = wsmall.tile([P, ND, HD], BF16, name="wvT")
            nc.sync.dma_start(wkT[:], wkT_ext[:])
            nc.scalar.dma_start(wvT[:], wvT_ext[:])
            # Chunk 0 is processed as two 256-wide QKV passes (separate tiles,
            # so every DMA destination stays contiguous): compute starts on the
            # first half while the rest of x and the q weights stream in.
            TC2 = TC // 2
            x0h = [
                slabp.tile([P, ND, TC2], BF16, tag="slab", name=f"x0h{i}")
                for i in range(2)
            ]
            wq_lo = wpool.tile([P, ND, HH], BF16, tag="bigw", name="wq_lo")
            wq_hi = wpool.tile([P, ND, HH], BF16, tag="bigw", name="wq_hi")

            def _wq_load(h):
                eng = (nc.sync, nc.scalar)[h % 2]
                dst = wq_lo if h < 4 else wq_hi
                eng.dma_start(dst[:, :, (h % 4) * P : (h % 4 + 1) * P], wqT_ext[h])

            hnd = ND // 2
            for qi, eng in enumerate((nc.sync, nc.scalar)):
                lo = qi * hnd
                eng.dma_start(x0h[0][:, lo : lo + hnd, :], x0_ext[0, :, lo : lo + hnd, :])
            _wq_load(0)
            _wq_load(1)
            for qi, eng in enumerate((nc.sync, nc.scalar)):
                lo = qi * hnd
                eng.dma_start(x0h[1][:, lo : lo + hnd, :], x0_ext[1, :, lo : lo + hnd, :])
            for h in range(2, HL):
                _wq_load(h)

            bq_sb = consts.tile([P, HL], F32)
            nc.sync.dma_start(bq_sb[:], bq_ext[:].rearrange("(o p) -> p o", p=P))
            bk_sb = consts.tile([P, 1], F32)
            nc.sync.dma_start(bk_sb[:], bk_ext[:].rearrange("(o p) -> p o", p=P))
            bv_sb = consts.tile([P, 1], F32)
            nc.sync.dma_start(bv_sb[:], bv_ext[:].rearrange("(o p) -> p o", p=P))
            bo_row = consts.tile([1, OL], BF16, name="bo_row")
            nc.scalar.dma_start(bo_row[:], bo_ext[None, :])
            bo_bc = consts.tile([P, OL], BF16)
            nc.gpsimd.partition_broadcast(bo_bc[:], bo_row[:])


            # Persistent attention operands.
            kT = bigp.tile([P, T], BF16, name="kT")               # [hd, t]
            vaug = bigp.tile([P, NT, HD + 1], BF16, name="vaug")  # [tk, kt, 129]
            nc.vector.memset(vaug[:, :, HD : HD + 1], 1.0)

            # AllGather buffers, one per t-chunk, partition-major:
            # cc_in[c][p, h, t] = attnT_c[p, h, t]; gathered over the 4 cores.
            cc_in = [
                dram.tile([P, HL, TC], BF16, name=f"cc_in{c}") for c in range(NTC)
            ]
            cc_g = [
                dram.tile([TPG, P, HL, TC], BF16, name=f"cc_g{c}") for c in range(NTC)
            ]
            # Chunk 3 ships in two 4-head halves.
            cc_in3 = [dram.tile([P, 4, TC], BF16, name=f"cc_in3{i}") for i in range(2)]
            cc_g3 = [
                dram.tile([TPG, P, 4, TC], BF16, name=f"cc_g3{i}") for i in range(2)
            ]

            wo_d = {}  # dc -> [P, ND, HH] tile, loaded during chunk-3 QKV

            def outproj_mms(c, gT, ots):
                for tt in range(TC // P):
                    for dc in range(OL // TC):
                        ps = psump.tile(
                            [P, TC], F32, tag="tr", bufs=2, name=f"pso{c}_{tt}_{dc}"
                        )
                        for i, ot in enumerate(ots):
                            nc.tensor.matmul(
                                ps[:],
                                gT[:, ot, tt * P : (tt + 1) * P],
                                wo_d[dc][:, ot, :],
                                start=(i == 0),
                                stop=(i == ND - 1),
                            )
                        osb = tmpp.tile(
                            [P, TC], F32, tag="osb", bufs=2, name=f"osb{c}{tt}{dc}"
                        )
                        nc.vector.tensor_tensor(
                            osb[:],
                            ps[:],
                            bo_bc[:, dc * TC : (dc + 1) * TC],
                            mybir.AluOpType.add,
                        )
                        nc.sync.dma_start(
                            out_ext[
                                c * TC + tt * P : c * TC + (tt + 1) * P,
                                dc * TC : (dc + 1) * TC,
                            ],
                            osb[:],
                        )

            def emit_outproj3():
                c = NTC - 1
                gT = slabp.tile([P, ND, TC], BF16, tag="slab", name="gT3")
                # 8KB-contiguous per-partition loads: one per (half, rank).
                for half in range(2):
                    eng = nc.sync if half == 0 else nc.scalar
                    for r in range(TPG):
                        eng.dma_start(
                            gT[:, r * HL + half * 4 : r * HL + half * 4 + 4, :],
                            cc_g3[half][r],
                        )
                # Accumulate first-half heads first (they arrive earlier).
                ots = [ot for ot in range(ND) if ot % HL < 4] + [
                    ot for ot in range(ND) if ot % HL >= 4
                ]
                outproj_mms(c, gT, ots)

            def emit_outproj(c):
                gT = slabp.tile([P, ND, TC], BF16, tag="slab", name=f"gT{c}")
                for r in range(TPG):
                    eng = nc.sync if r % 2 == 0 else nc.scalar
                    eng.dma_start(gT[:, r * HL : (r + 1) * HL, :], cc_g[c][r])
                outproj_mms(c, gT, list(range(ND)))

            # ------------- per t-chunk: QKV proj + attention ------------------
            for c in range(NTC):
                qTc = qtcp.tile([P, HL, TC], BF16, tag="qTc", name=f"qTc{c}")

                # k first, then v, then q heads: k/v weights are tiny and land
                # first, so compute starts as early as possible.
                def qkv_pass(xsrc, t0, tw):
                    for ot in [HL, HL + 1] + list(range(HL)):
                        ps = psump.tile(
                            [P, tw], F32, tag="mm512", bufs=2,
                            name=f"psqkv{c}_{ot}_{t0}",
                        )
                        for dt in range(ND):
                            if ot < 4:
                                lhsT = wq_lo[:, dt, ot * P : (ot + 1) * P]
                            elif ot < HL:
                                lhsT = wq_hi[:, dt, (ot - 4) * P : (ot - 3) * P]
                            elif ot == HL:
                                lhsT = wkT[:, dt, :]
                            else:
                                lhsT = wvT[:, dt, :]
                            nc.tensor.matmul(
                                ps[:],
                                lhsT,
                                xsrc[:, dt, :],
                                start=(dt == 0),
                                stop=(dt == ND - 1),
                            )
                        if ot < HL:
                            nc.vector.tensor_scalar_add(
                                qTc[:, ot, t0 : t0 + tw], ps[:], bq_sb[:, ot : ot + 1]
                            )
                        elif ot == HL:
                            nc.vector.tensor_scalar_add(
                                kT[:, c * TC + t0 : c * TC + t0 + tw], ps[:], bk_sb[:]
                            )
                        else:
                            vt = tmpp.tile(
                                [P, tw], BF16, tag="vt", bufs=1, name=f"vt{c}{t0}"
                            )
                            nc.vector.tensor_scalar_add(vt[:], ps[:], bv_sb[:])
                            pstv = psump.tile(
                                [P, tw], BF16, tag="tr", bufs=2, name=f"pstv{c}{t0}"
                            )
                            for j in range(tw // P):
                                nc.tensor.transpose(
                                    pstv[:, j * P : (j + 1) * P],
                                    vt[:, j * P : (j + 1) * P],
                                    ident[:],
                                )
                            nc.vector.tensor_copy(
                                vaug[
                                    :, c * 4 + t0 // P : c * 4 + (t0 + tw) // P, 0:HD
                                ],
                                pstv[:].rearrange("p (g t) -> p g t", g=tw // P),
                            )

                if c == 0:
                    qkv_pass(x0h[0], 0, TC2)
                    qkv_pass(x0h[1], TC2, TC2)
                else:
                    qkv_pass(_CACHE[f"_xT{c}"], 0, TC)

                if c == NTC - 1:
                    # wo halves reuse the wq buffers; the tile deps make each
                    # load start once the matching wq subtile is dead.
                    wo_d[0] = wpool.tile([P, ND, HH], BF16, tag="bigw", name="wo0")
                    wo_d[1] = wpool.tile([P, ND, HH], BF16, tag="bigw", name="wo1")
                    nc.sync.dma_start(wo_d[0][:], woT_ext[0])
                    nc.sync.dma_start(wo_d[1][:], woT_ext[1])

                if c + 1 < NTC:
                    # Prefetch next chunk's x before this chunk's attention so
                    # nothing with a long semaphore wait sits ahead of it.
                    xTn = slabp.tile([P, ND, TC], BF16, tag="slab", name=f"xT{c+1}")
                    load_xT(xTn, c + 1)
                    _CACHE[f"_xT{c+1}"] = xTn

                # Attention for all local heads on this q-chunk.  The per-head
                # normalize (DVE) + transpose (TensorE) chain is deferred into
                # the NEXT head's score phase so it never gates the PE.
                attnT_c = atcp.tile([P, HL, TC], BF16, tag="atc", name=f"attnT{c}")
                nkt = (c + 1) * (TC // P)  # causal: k-tiles 0..nkt-1

                def emit_psta(ph, sbs):
                    psta = psump.tile(
                        [P, 4 * P], BF16, tag="tr", bufs=2, name=f"psta{c}{ph}"
                    )
                    for qt in range(TC // P):
                        nc.tensor.transpose(
                            psta[:, qt * P : (qt + 1) * P], sbs[qt][:], ident[:]
                        )
                    nc.vector.tensor_copy(attnT_c[:, ph, :], psta[:])

                pending = None
                for h in range(HL):
                    att_ps = [
                        psump.tile(
                            [P, HD + 1], F32, tag="attn", bufs=4,
                            name=f"att{c}_{h}_{qt}",
                        )
                        for qt in range(TC // P)
                    ]
                    for kt in range(nkt):
                        # Band tiles only need columns tq >= (kt-4c)*128; after
                        # trimming, the causal pattern is always the diagonal.
                        off = (kt - (nkt - 4)) * P if kt >= nkt - 4 else 0
                        ne = TC - off
                        ps_s = psump.tile(
                            [P, TC], F32, tag="mm512", bufs=2, name=f"pss{c}_{h}_{kt}"
                        )
                        nc.tensor.matmul(
                            ps_s[:, :ne],
                            kT[:, kt * P : (kt + 1) * P],
                            qTc[:, h, off:TC],
                            start=True,
                            stop=True,
                        )
                        if kt == 1 and pending is not None:
                            # Deferred by a full k-tile so the previous head's
                            # DVE normalize is long done when the PE transposes.
                            emit_psta(*pending)
                            pending = None
                        es = esbp.tile([P, TC], BF16, tag="esb", name=f"es{c}_{h}_{kt}")
                        nc.scalar.activation(
                            es[:, :ne],
                            ps_s[:, :ne],
                            mybir.ActivationFunctionType.Exp,
                            scale=SCALE,
                        )
                        if kt >= nkt - 4:
                            # Zero weights where k > q (pure diagonal after trim).
                            nc.vector.tensor_tensor(
                                es[:, :ne], es[:, :ne], mask0[:, :ne],
                                mybir.AluOpType.mult,
                            )
                        for qt in range(TC // P):
                            tqi = c * (TC // P) + qt
                            if kt > tqi:
                                continue
                            nc.tensor.matmul(
                                att_ps[qt][:],
                                es[:, qt * P - off : (qt + 1) * P - off],
                                vaug[:, kt, :],
                                start=(kt == 0),
                                stop=(kt == tqi),
                            )
                    sbs = []
                    for qt in range(TC // P):
                        recip = tmpp.tile([P, 1], F32, tag="recip", bufs=2, name=f"rc{c}{h}{qt}")
                        nc.vector.reciprocal(recip[:], att_ps[qt][:, HD : HD + 1])
                        attn_sb = tmpp.tile(
                            [P, P], BF16, tag="attn_sb", bufs=8, name=f"asb{c}{h}{qt}"
                        )
                        nc.vector.tensor_scalar_mul(
                            attn_sb[:], att_ps[qt][:, 0:HD], recip[:]
                        )
                        sbs.append(attn_sb)
                    pending = (h, sbs)
                    # Flush before anything that READS attnT_c (the cc_in
                    # stores at h==3 of the last chunk and at chunk end).
                    if h == HL - 1 or (c == NTC - 1 and h == 3):
                        emit_psta(*pending)
                        pending = None
                    if c == NTC - 1 and h == 3:
                        nc.gpsimd.dma_start(cc_in3[0][:], attnT_c[:, 0:4, :])
                        nc.gpsimd.collective_compute(
                            "AllGather",
                            mybir.AluOpType.bypass,
                            replica_groups=[[0, 1, 2, 3], [4, 5, 6, 7]],
                            ins=[cc_in3[0][:, :, :].opt()],
                            outs=[cc_g3[0][:, :, :, :].opt()],
                        )
                        emit_outproj(0)
                    if c == NTC - 1 and h == 5:
                        emit_outproj(1)
                    if c == NTC - 1 and h == 7:
                        emit_outproj(2)

                # Ship this chunk's attn^T and AllGather it within the group.
                if c < NTC - 1:
                    nc.gpsimd.dma_start(cc_in[c][:], attnT_c[:])
                    nc.gpsimd.collective_compute(
                        "AllGather",
                        mybir.AluOpType.bypass,
                        replica_groups=[[0, 1, 2, 3], [4, 5, 6, 7]],
                        ins=[cc_in[c][:, :, :].opt()],
                        outs=[cc_g[c][:, :, :, :].opt()],
                    )
                else:
                    nc.gpsimd.dma_start(cc_in3[1][:], attnT_c[:, 4:HL, :])
                    nc.gpsimd.collective_compute(
                        "AllGather",
                        mybir.AluOpType.bypass,
                        replica_groups=[[0, 1, 2, 3], [4, 5, 6, 7]],
                        ins=[cc_in3[1][:, :, :].opt()],
                        outs=[cc_g3[1][:, :, :, :].opt()],
                    )
                    emit_outproj3()

            for c in range(NTC):
                _CACHE.pop(f"_xT{c}", None)

    nc.compile()
    return nc


def _pack(mT, nd, width):
    """[nd*128, width] row-major -> [128, nd, width] partition-major."""
    return np.ascontiguousarray(mT.reshape(nd, P, width).transpose(1, 0, 2))


def kernel(x, wq_w, wq_b, wk_w, wk_b, wv_w, wv_b, wo_w, wo_b):
    global LAST_RESULT
    if "nc" not in _CACHE:
        _CACHE["nc"] = build_nc()
    nc = _CACHE["nc"]

    f32 = np.float32
    bf16 = ml_dtypes.bfloat16
    x = np.asarray(x, f32)
    wq_w = np.asarray(wq_w, f32)
    wk_w = np.asarray(wk_w, f32)
    wv_w = np.asarray(wv_w, f32)
    wo_w = np.asarray(wo_w, f32)

    # Host-side cast + transpose + pack into partition-major tile layout.
    # xT[c, p, nd, t] = x[b, c*TC+t, nd*P+p]
    xT_b = [
        np.ascontiguousarray(
            x[b].astype(bf16).reshape(NTC, TC, ND, P).transpose(0, 3, 2, 1)
        )
        for b in range(B)
    ]
    wkT = _pack(wk_w.astype(bf16).T, ND, HD)
    wvT = _pack(wv_w.astype(bf16).T, ND, HD)
    wqT_g = {}
    woT_g = {}
    for g in range(TPG):
        sl = slice(OL * g, OL * (g + 1))
        # wqT[h, p, nd, o] = wq[sl][h*P+o, nd*P+p]
        wqT_g[g] = np.ascontiguousarray(
            wq_w[sl].astype(bf16).reshape(HL, P, ND, P).transpose(0, 3, 2, 1)
        )
        # woT[half, p, nd, o] = wo[sl][half*HH+o, nd*P+p]
        woT_g[g] = np.ascontiguousarray(
            wo_w[sl].astype(bf16).reshape(2, HH, ND, P).transpose(0, 3, 2, 1)
        )

    # x0[half, p, nd, t] = x[b, half*256+t, nd*P+p] (chunk 0 only)
    x0_b = [
        np.ascontiguousarray(
            x[b, :TC].astype(bf16).reshape(2, TC // 2, ND, P).transpose(0, 3, 2, 1)
        )
        for b in range(B)
    ]

    in_maps = []
    for c in range(NCORES):
        b, g = divmod(c, TPG)
        sl = slice(OL * g, OL * (g + 1))
        in_maps.append(
            {
                "xT": xT_b[b],
                "x0": x0_b[b],
                "wqT": wqT_g[g],
                "bq": np.ascontiguousarray(np.asarray(wq_b, f32)[sl]),
                "wkT": wkT,
                "bk": np.ascontiguousarray(np.asarray(wk_b, f32)),
                "wvT": wvT,
                "bv": np.ascontiguousarray(np.asarray(wv_b, f32)),
                "woT": woT_g[g],
                "bo": np.ascontiguousarray(np.asarray(wo_b, f32)[sl].astype(bf16)),
            }
        )

    import os

    trace = bool(os.environ.get("ATTN_TRACE"))
    res = run_bass_kernel_spmd(
        nc,
        in_maps,
        core_ids=list(range(NCORES)),
        trace=trace,
        trace_cores=list(range(NCORES)) if trace else None,
    )
    LAST_RESULT = res

    out = np.empty((B, T, D), dtype=f32)
    for c in range(NCORES):
        b, g = divmod(c, TPG)
        out[b, :, OL * g : OL * (g + 1)] = res.results[c]["out"]
    return out


# revision 28
# speedup vs baseline: 1.0498x; 1.0417x over previous
"""Distributed MQA causal attention for TRN2 (8 NeuronCores).

Sharding: 8 cores = 2 (batch) x 4 (head-group tensor parallel).
Core c handles batch b=c//4, head group g=c%4 (8 heads, o-slice of 1024).
KV projection is replicated within a batch group.  After attention, the
per-core attn^T chunks are AllGather-ed (groups of 4) and each core computes
a 1024-wide column slice of the output projection.

All operands are pre-transposed, cast to bf16, and PACKED INTO PARTITION-MAJOR
TILE LAYOUT on the host, so every device load is a plain DMA with 8-32KB
contiguous runs per partition.  Loads are spread across the two HWDGE queues
(sync + scalar); stores that wait on long-running compute (attn^T -> cc_in)
go on the gpsimd SWDGE queue so they never head-of-line-block a load.
All matmuls run in bf16 with f32 accumulation in PSUM.
"""

import numpy as np
import ml_dtypes

import concourse.bass as bass
import concourse.mybir as mybir
import concourse.tile as tile
from concourse import bacc
from concourse.bass_utils import run_bass_kernel_spmd
from concourse.masks import make_identity

# Problem shape (hardcoded; kernel.py must be self-contained).
B, T, D = 2, 2048, 4096
H, HD = 32, 128
NCORES, TPG = 8, 4
HL = H // TPG            # 8 local heads per core
OL = HL * HD             # 1024 local q/o dims per core
P = 128
TC = 512                 # t-chunk width (moving-dim of the big GEMMs)
NTC = T // TC            # 4
ND = D // P              # 32 contraction tiles for D
NT = T // P              # 16 k-tiles
HH = OL // 2             # 512 = 4 heads
SCALE = float(1.0 / np.sqrt(HD))

BF16 = mybir.dt.bfloat16
F32 = mybir.dt.float32

_CACHE = {}
LAST_RESULT = None  # BassKernelResults of the most recent run (for test harness)


def build_nc():
    nc = bacc.Bacc(None, target_bir_lowering=False, num_devices=NCORES)

    # Host-packed bf16 operands (partition-major tile layout).
    xT_ext = nc.declare_dram_parameter("xT", [NTC, P, ND, TC], BF16, isOutput=False)
    wqT_ext = nc.declare_dram_parameter("wqT", [HL, P, ND, P], BF16, isOutput=False)
    wkT_ext = nc.declare_dram_parameter("wkT", [P, ND, HD], BF16, isOutput=False)
    wvT_ext = nc.declare_dram_parameter("wvT", [P, ND, HD], BF16, isOutput=False)
    woT_ext = nc.declare_dram_parameter("woT", [2, P, ND, HH], BF16, isOutput=False)
    bq_ext = nc.declare_dram_parameter("bq", [OL], F32, isOutput=False)
    bk_ext = nc.declare_dram_parameter("bk", [HD], F32, isOutput=False)
    bv_ext = nc.declare_dram_parameter("bv", [HD], F32, isOutput=False)
    bo_ext = nc.declare_dram_parameter("bo", [OL], BF16, isOutput=False)
    out_ext = nc.declare_dram_parameter("out", [T, OL], F32, isOutput=True)

    NPIECE = 4  # xT chunk load pieces per queue

    with tile.TileContext(nc) as tc:
        with (
            tc.tile_pool(name="consts", bufs=1) as consts,
            tc.tile_pool(name="wpool", bufs=2) as wpool,
            tc.tile_pool(name="wsmall", bufs=1) as wsmall,
            tc.tile_pool(name="slab", bufs=2) as slabp,
            tc.tile_pool(name="big", bufs=1) as bigp,
            tc.tile_pool(name="qtc", bufs=2) as qtcp,
            tc.tile_pool(name="atc", bufs=1) as atcp,
            tc.tile_pool(name="esb", bufs=3) as esbp,
            tc.tile_pool(name="tmp", bufs=3) as tmpp,
            tc.tile_pool(name="psum", bufs=1, space="PSUM") as psump,
            tc.tile_pool(name="dram", bufs=1, space="DRAM") as dram,
        ):
            # ---- Small constants
            ident = consts.tile([P, P], BF16)
            make_identity(nc, ident[:])
            # Diagonal causal 0/1 mask: mask0[x, y] = 1 if y >= x else 0.
            # Band S-tiles are column-trimmed so every one reduces to this.
            mask0 = consts.tile([P, TC], BF16, name="mask0")
            nc.gpsimd.memset(mask0[:], 1.0)
            nc.gpsimd.affine_select(
                out=mask0[:],
                in_=mask0[:],
                pattern=[[1, TC]],
                compare_op=mybir.AluOpType.is_ge,
                fill=0.0,
                base=0,
                channel_multiplier=-1,
            )

            def load_xT(xT, c):
                # 2*NPIECE interleaved pieces across both HWDGE queues.
                w = ND // (2 * NPIECE)
                for i in range(NPIECE):
                    for qi, eng in enumerate((nc.sync, nc.scalar)):
                        lo = (2 * i + qi) * w
                        eng.dma_start(
                            xT[:, lo : lo + w, :], xT_ext[c, :, lo : lo + w, :]
                        )

            # Startup loads are spread over THREE queues (sync, scalar, and
            # the otherwise-idle gpsimd SWDGE) so the k-projection can start
            # within a few microseconds.  gpsimd is only safe here, before any
            # cc_in store is enqueued on it.
            wkT = wsmall.tile([P, ND, HD], BF16, name="wkT")
            wvT = wsmall.tile([P, ND, HD], BF16, name="wvT")
            nc.sync.dma_start(wkT[:], wkT_ext[:])
            nc.scalar.dma_start(wvT[:], wvT_ext[:])
            xT0 = slabp.tile([P, ND, TC], BF16, tag="slab", name="xT0")
            w0 = ND // 8
            for i in range(8):
                lo = i * w0
                eng = nc.sync if i % 2 == 0 else nc.scalar
                eng.dma_start(xT0[:, lo : lo + w0, :], xT_ext[0, :, lo : lo + w0, :])

            bq_sb = consts.tile([P, HL], F32)
            nc.sync.dma_start(bq_sb[:], bq_ext[:].rearrange("(o p) -> p o", p=P))
            bk_sb = consts.tile([P, 1], F32)
            nc.sync.dma_start(bk_sb[:], bk_ext[:].rearrange("(o p) -> p o", p=P))
            bv_sb = consts.tile([P, 1], F32)
            nc.sync.dma_start(bv_sb[:], bv_ext[:].rearrange("(o p) -> p o", p=P))
            bo_row = consts.tile([1, OL], BF16, name="bo_row")
            nc.scalar.dma_start(bo_row[:], bo_ext[None, :])
            bo_bc = consts.tile([P, OL], BF16)
            nc.gpsimd.partition_broadcast(bo_bc[:], bo_row[:])

            # wq in two 4-head subtiles; wo reuses their buffers at chunk 3.
            wq_lo = wpool.tile([P, ND, HH], BF16, tag="bigw", name="wq_lo")
            wq_hi = wpool.tile([P, ND, HH], BF16, tag="bigw", name="wq_hi")
            for h in range(HL):
                eng = (nc.sync, nc.scalar)[h % 2]
                dst = wq_lo if h < 4 else wq_hi
                eng.dma_start(dst[:, :, (h % 4) * P : (h % 4 + 1) * P], wqT_ext[h])

            # Persistent attention operands.
            kT = bigp.tile([P, T], BF16, name="kT")               # [hd, t]
            vaug = bigp.tile([P, NT, HD + 1], BF16, name="vaug")  # [tk, kt, 129]
            nc.vector.memset(vaug[:, :, HD : HD + 1], 1.0)

            # AllGather buffers, one per t-chunk, partition-major:
            # cc_in[c][p, h, t] = attnT_c[p, h, t]; gathered over the 4 cores.
            cc_in = [
                dram.tile([P, HL, TC], BF16, name=f"cc_in{c}") for c in range(NTC)
            ]
            cc_g = [
                dram.tile([TPG, P, HL, TC], BF16, name=f"cc_g{c}") for c in range(NTC)
            ]
            # Chunk 3 ships in two 4-head halves.
            cc_in3 = [dram.tile([P, 4, TC], BF16, name=f"cc_in3{i}") for i in range(2)]
            cc_g3 = [
                dram.tile([TPG, P, 4, TC], BF16, name=f"cc_g3{i}") for i in range(2)
            ]

            wo_d = {}  # dc -> [P, ND, HH] tile, loaded during chunk-3 QKV

            def outproj_mms(c, gT, ots):
                for tt in range(TC // P):
                    for dc in range(OL // TC):
                        ps = psump.tile(
                            [P, TC], F32, tag="tr", bufs=2, name=f"pso{c}_{tt}_{dc}"
                        )
                        for i, ot in enumerate(ots):
                            nc.tensor.matmul(
                                ps[:],
                                gT[:, ot, tt * P : (tt + 1) * P],
                                wo_d[dc][:, ot, :],
                                start=(i == 0),
                                stop=(i == ND - 1),
                            )
                        osb = tmpp.tile(
                            [P, TC], F32, tag="osb", bufs=2, name=f"osb{c}{tt}{dc}"
                        )
                        nc.vector.tensor_tensor(
                            osb[:],
                            ps[:],
                            bo_bc[:, dc * TC : (dc + 1) * TC],
                            mybir.AluOpType.add,
                        )
                        nc.sync.dma_start(
                            out_ext[
                                c * TC + tt * P : c * TC + (tt + 1) * P,
                                dc * TC : (dc + 1) * TC,
                            ],
                            osb[:],
                        )

            def emit_outproj3():
                c = NTC - 1
                gT = slabp.tile([P, ND, TC], BF16, tag="slab", name="gT3")
                # 8KB-contiguous per-partition loads: one per (half, rank).
                for half in range(2):
                    eng = nc.sync if half == 0 else nc.scalar
                    for r in range(TPG):
                        eng.dma_start(
                            gT[:, r * HL + half * 4 : r * HL + half * 4 + 4, :],
                            cc_g3[half][r],
                        )
                # Accumulate first-half heads first (they arrive earlier).
                ots = [ot for ot in range(ND) if ot % HL < 4] + [
                    ot for ot in range(ND) if ot % HL >= 4
                ]
                outproj_mms(c, gT, ots)

            def emit_outproj(c):
                gT = slabp.tile([P, ND, TC], BF16, tag="slab", name=f"gT{c}")
                for r in range(TPG):
                    eng = nc.sync if r % 2 == 0 else nc.scalar
                    eng.dma_start(gT[:, r * HL : (r + 1) * HL, :], cc_g[c][r])
                outproj_mms(c, gT, list(range(ND)))

            # ------------- per t-chunk: QKV proj + attention ------------------
            for c in range(NTC):
                if c == 0:
                    xT = xT0
                else:
                    xT = _CACHE[f"_xT{c}"]  # prefetched below

                qTc = qtcp.tile([P, HL, TC], BF16, tag="qTc", name=f"qTc{c}")

                # k first, then v, then q heads: k/v weights are tiny and land
                # first, so compute starts as early as possible.
                for ot in [HL, HL + 1] + list(range(HL)):
                    ps = psump.tile(
                        [P, TC], F32, tag="mm512", bufs=2, name=f"psqkv{c}_{ot}"
                    )
                    for dt in range(ND):
                        if ot < 4:
                            lhsT = wq_lo[:, dt, ot * P : (ot + 1) * P]
                        elif ot < HL:
                            lhsT = wq_hi[:, dt, (ot - 4) * P : (ot - 3) * P]
                        elif ot == HL:
                            lhsT = wkT[:, dt, :]
                        else:
                            lhsT = wvT[:, dt, :]
                        nc.tensor.matmul(
                            ps[:],
                            lhsT,
                            xT[:, dt, :],
                            start=(dt == 0),
                            stop=(dt == ND - 1),
                        )
                    if ot < HL:
                        nc.vector.tensor_scalar_add(
                            qTc[:, ot, :], ps[:], bq_sb[:, ot : ot + 1]
                        )
                    elif ot == HL:
                        nc.vector.tensor_scalar_add(
                            kT[:, c * TC : (c + 1) * TC], ps[:], bk_sb[:]
                        )
                    else:
                        vt = tmpp.tile([P, TC], BF16, tag="vt", bufs=1, name=f"vt{c}")
                        nc.vector.tensor_scalar_add(vt[:], ps[:], bv_sb[:])
                        pstv = psump.tile(
                            [P, 4 * P], BF16, tag="tr", bufs=2, name=f"pstv{c}"
                        )
                        for j in range(TC // P):
                            nc.tensor.transpose(
                                pstv[:, j * P : (j + 1) * P],
                                vt[:, j * P : (j + 1) * P],
                                ident[:],
                            )
                        nc.vector.tensor_copy(
                            vaug[:, c * (TC // P) : (c + 1) * (TC // P), 0:HD],
                            pstv[:].rearrange("p (g t) -> p g t", g=4),
                        )

                if c == NTC - 1:
                    # wo halves reuse the wq buffers; the tile deps make each
                    # load start once the matching wq subtile is dead.
                    wo_d[0] = wpool.tile([P, ND, HH], BF16, tag="bigw", name="wo0")
                    wo_d[1] = wpool.tile([P, ND, HH], BF16, tag="bigw", name="wo1")
                    nc.sync.dma_start(wo_d[0][:], woT_ext[0])
                    nc.sync.dma_start(wo_d[1][:], woT_ext[1])

                if c + 1 < NTC:
                    # Prefetch next chunk's x before this chunk's attention so
                    # nothing with a long semaphore wait sits ahead of it.
                    xTn = slabp.tile([P, ND, TC], BF16, tag="slab", name=f"xT{c+1}")
                    load_xT(xTn, c + 1)
                    _CACHE[f"_xT{c+1}"] = xTn

                # Attention for all local heads on this q-chunk.  The per-head
                # normalize (DVE) + transpose (TensorE) chain is deferred into
                # the NEXT head's score phase so it never gates the PE.
                attnT_c = atcp.tile([P, HL, TC], BF16, tag="atc", name=f"attnT{c}")
                nkt = (c + 1) * (TC // P)  # causal: k-tiles 0..nkt-1

                def emit_psta(ph, sbs):
                    psta = psump.tile(
                        [P, 4 * P], BF16, tag="tr", bufs=2, name=f"psta{c}{ph}"
                    )
                    for qt in range(TC // P):
                        nc.tensor.transpose(
                            psta[:, qt * P : (qt + 1) * P], sbs[qt][:], ident[:]
                        )
                    nc.vector.tensor_copy(attnT_c[:, ph, :], psta[:])

                pending = None
                for h in range(HL):
                    att_ps = [
                        psump.tile(
                            [P, HD + 1], F32, tag="attn", bufs=4,
                            name=f"att{c}_{h}_{qt}",
                        )
                        for qt in range(TC // P)
                    ]
                    for kt in range(nkt):
                        # Band tiles only need columns tq >= (kt-4c)*128; after
                        # trimming, the causal pattern is always the diagonal.
                        off = (kt - (nkt - 4)) * P if kt >= nkt - 4 else 0
                        ne = TC - off
                        ps_s = psump.tile(
                            [P, TC], F32, tag="mm512", bufs=2, name=f"pss{c}_{h}_{kt}"
                        )
                        nc.tensor.matmul(
                            ps_s[:, :ne],
                            kT[:, kt * P : (kt + 1) * P],
                            qTc[:, h, off:TC],
                            start=True,
                            stop=True,
                        )
                        if kt == 1 and pending is not None:
                            # Deferred by a full k-tile so the previous head's
                            # DVE normalize is long done when the PE transposes.
                            emit_psta(*pending)
                            pending = None
                        es = esbp.tile([P, TC], BF16, tag="esb", name=f"es{c}_{h}_{kt}")
                        nc.scalar.activation(
                            es[:, :ne],
                            ps_s[:, :ne],
                            mybir.ActivationFunctionType.Exp,
                            scale=SCALE,
                        )
                        if kt >= nkt - 4:
                            # Zero weights where k > q (pure diagonal after trim).
                            nc.vector.tensor_tensor(
                                es[:, :ne], es[:, :ne], mask0[:, :ne],
                                mybir.AluOpType.mult,
                            )
                        for qt in range(TC // P):
                            tqi = c * (TC // P) + qt
                            if kt > tqi:
                                continue
                            nc.tensor.matmul(
                                att_ps[qt][:],
                                es[:, qt * P - off : (qt + 1) * P - off],
                                vaug[:, kt, :],
                                start=(kt == 0),
                                stop=(kt == tqi),
                            )
                    sbs = []
                    for qt in range(TC // P):
                        recip = tmpp.tile([P, 1], F32, tag="recip", bufs=2, name=f"rc{c}{h}{qt}")
                        nc.vector.reciprocal(recip[:], att_ps[qt][:, HD : HD + 1])
                        attn_sb = tmpp.tile(
                            [P, P], BF16, tag="attn_sb", bufs=8, name=f"asb{c}{h}{qt}"
                        )
                        nc.vector.tensor_scalar_mul(
                            attn_sb[:], att_ps[qt][:, 0:HD], recip[:]
                        )
                        sbs.append(attn_sb)
                    pending = (h, sbs)
                    # Flush before anything that READS attnT_c (the cc_in
                    # stores at h==3 of the last chunk and at chunk end).
                    if h == HL - 1 or (c == NTC - 1 and h == 3):
                        emit_psta(*pending)
                        pending = None
                    if c == NTC - 1 and h == 3:
                        nc.gpsimd.dma_start(cc_in3[0][:], attnT_c[:, 0:4, :])
                        nc.gpsimd.collective_compute(
                            "AllGather",
                            mybir.AluOpType.bypass,
                            replica_groups=[[0, 1, 2, 3], [4, 5, 6, 7]],
                            ins=[cc_in3[0][:, :, :].opt()],
                            outs=[cc_g3[0][:, :, :, :].opt()],
                        )
                        emit_outproj(0)
                    if c == NTC - 1 and h == 5:
                        emit_outproj(1)
                    if c == NTC - 1 and h == 7:
                        emit_outproj(2)

                # Ship this chunk's attn^T and AllGather it within the group.
                if c < NTC - 1:
                    nc.gpsimd.dma_start(cc_in[c][:], attnT_c[:])
                    nc.gpsimd.collective_compute(
                        "AllGather",
                        mybir.AluOpType.bypass,
                        replica_groups=[[0, 1, 2, 3], [4, 5, 6, 7]],
                        ins=[cc_in[c][:, :, :].opt()],
                        outs=[cc_g[c][:, :, :, :].opt()],
                    )
                else:
                    nc.gpsimd.dma_start(cc_in3[1][:], attnT_c[:, 4:HL, :])
                    nc.gpsimd.collective_compute(
                        "AllGather",
                        mybir.AluOpType.bypass,
                        replica_groups=[[0, 1, 2, 3], [4, 5, 6, 7]],
                        ins=[cc_in3[1][:, :, :].opt()],
                        outs=[cc_g3[1][:, :, :, :].opt()],
                    )
                    emit_outproj3()

            for c in range(NTC):
                _CACHE.pop(f"_xT{c}", None)

    nc.compile()
    return nc


def _pack(mT, nd, width):
    """[nd*128, width] row-major -> [128, nd, width] partition-major."""
    return np.ascontiguousarray(mT.reshape(nd, P, width).transpose(1, 0, 2))


def kernel(x, wq_w, wq_b, wk_w, wk_b, wv_w, wv_b, wo_w, wo_b):
    global LAST_RESULT
    if "nc" not in _CACHE:
        _CACHE["nc"] = build_nc()
    nc = _CACHE["nc"]

    f32 = np.float32
    bf16 = ml_dtypes.bfloat16
    x = np.asarray(x, f32)
    wq_w = np.asarray(wq_w, f32)
    wk_w = np.asarray(wk_w, f32)
    wv_w = np.asarray(wv_w, f32)
    wo_w = np.asarray(wo_w, f32)

    # Host-side cast + transpose + pack into partition-major tile layout.
    # xT[c, p, nd, t] = x[b, c*TC+t, nd*P+p]
    xT_b = [
        np.ascontiguousarray(
            x[b].astype(bf16).reshape(NTC, TC, ND, P).transpose(0, 3, 2, 1)
        )
        for b in range(B)
    ]
    wkT = _pack(wk_w.astype(bf16).T, ND, HD)
    wvT = _pack(wv_w.astype(bf16).T, ND, HD)
    wqT_g = {}
    woT_g = {}
    for g in range(TPG):
        sl = slice(OL * g, OL * (g + 1))
        # wqT[h, p, nd, o] = wq[sl][h*P+o, nd*P+p]
        wqT_g[g] = np.ascontiguousarray(
            wq_w[sl].astype(bf16).reshape(HL, P, ND, P).transpose(0, 3, 2, 1)
        )
        # woT[half, p, nd, o] = wo[sl][half*HH+o, nd*P+p]
        woT_g[g] = np.ascontiguousarray(
            wo_w[sl].astype(bf16).reshape(2, HH, ND, P).transpose(0, 3, 2, 1)
        )

    in_maps = []
    for c in range(NCORES):
        b, g = divmod(c, TPG)
        sl = slice(OL * g, OL * (g + 1))
        in_maps.append(
            {
                "xT": xT_b[b],
                "wqT": wqT_g[g],
                "bq": np.ascontiguousarray(np.asarray(wq_b, f32)[sl]),
                "wkT": wkT,
                "bk": np.ascontiguousarray(np.asarray(wk_b, f32)),
                "wvT": wvT,
                "bv": np.ascontiguousarray(np.asarray(wv_b, f32)),
                "woT": woT_g[g],
                "bo": np.ascontiguousarray(np.asarray(wo_b, f32)[sl].astype(bf16)),
            }
        )

    import os

    trace = bool(os.environ.get("ATTN_TRACE"))
    res = run_bass_kernel_spmd(
        nc,
        in_maps,
        core_ids=list(range(NCORES)),
        trace=trace,
        trace_cores=list(range(NCORES)) if trace else None,
    )
    LAST_RESULT = res

    out = np.empty((B, T, D), dtype=f32)
    for c in range(NCORES):
        b, g = divmod(c, TPG)
        out[b, :, OL * g : OL * (g + 1)] = res.results[c]["out"]
    return out
